# revision 32
# baseline (speedup 1.0000x reference)
"""GPT (4-layer, C=512, H=8, T=1024, B=2, V=50257, LoRA r=8) on 8 trn2 cores.

Sharding: every core owns global token tile c (128 tokens) of BOTH batches.
The two batch streams are software-pipelined inside each layer so the
per-batch 8-rank KV AllGather latency hides under the other batch's compute:
    qkv(i,b0) -> fire AG0 ; mlp(i-1,b1) ; qkv(i,b1) -> fire AG1 ;
    attn(i,b0) ; attn(i,b1) ; mlp(i,b0) ; [mlp(i,b1) deferred]
Head: vocab-sharded (6400-padded shard per core), out partition = vocab,
bf16 logits, 50 contiguous 512KB output DMAs; host upcasts/transposes.
"""
import math
import numpy as np
import ml_dtypes

import concourse.bass as bass
import concourse.bacc as bacc
import concourse.tile as tile
import concourse.mybir as mybir
from concourse import bass_utils

BF16 = mybir.dt.bfloat16
F32 = mybir.dt.float32
AF = mybir.ActivationFunctionType

L, H, C, V, B, T = 4, 8, 512, 50257, 2, 1024
R = 8
NCORES = 8
NF = C // 128        # 4 feature tiles
HD = C // H          # 64 head dim
VC = 6283            # true vocab shard (8*6283 = 50264 >= 50257)
VCP = 6400           # padded shard: 50 slices of 128
NEG = -1.0e9

_CACHE = {}


def build_nc(debug=False):
    nc = bacc.Bacc("TRN2", target_bir_lowering=False, debug=False,
                   num_devices=NCORES)
    d = {}
    def inp(name, shape, dt):
        d[name] = nc.dram_tensor(name, shape, dt, kind="ExternalInput").ap()
    inp("x0", [2, 128, C], F32)        # [batch, own 128 tokens, C]
    inp("maskT", [128, 8, 128], BF16)  # [kk, kc, qq] causal add-mask
    inp("ident", [128, 128], BF16)
    # weights host-pre-rearranged to [p, f, n] so loads are contiguous
    inp("aw", [L, 128, NF, 3 * C], BF16)
    inp("ala", [L, 128, NF, R], BF16)
    inp("alb", [L, R, 3 * C], BF16)    # *4.0, q-cols pre-scaled
    inp("pw", [L, 128, NF, C], BF16)
    inp("pla", [L, 128, NF, R], BF16)
    inp("plb", [L, R, C], BF16)        # *4.0
    inp("fw", [L, 128, NF, 4 * C], BF16)
    inp("mw", [L, 128, 16, C], BF16)
    inp("hw", [C, VCP], BF16)          # head shard (rank-dep, zero-padded)
    y_d = nc.dram_tensor("y", [VCP, 2 * T], BF16, kind="ExternalOutput").ap()
    if debug:
        xdbg = nc.dram_tensor("xdbg", [L, 2, 128, C], F32,
                              kind="ExternalOutput").ap()

    with tile.TileContext(nc) as tc:
        with (
            tc.tile_pool(name="persist", bufs=1) as pp,
            tc.tile_pool(name="wts", bufs=1) as wp,
            tc.tile_pool(name="acts", bufs=1) as ap_,
            tc.tile_pool(name="acts3", bufs=3) as ap3,
            tc.tile_pool(name="stats", bufs=3) as sp,
            tc.tile_pool(name="dram", bufs=2, space="DRAM") as dp,
            tc.tile_pool(name="psu", bufs=8, space="PSUM") as psu,
        ):
            ident = pp.tile([128, 128], BF16)
            nc.sync.dma_start(ident[:], d["ident"][:])
            zt = pp.tile([128, 1], F32)
            nc.vector.memset(zt[:], 0.0)
            eps = pp.tile([128, 1], F32)
            nc.vector.memset(eps[:], 1e-5)
            maskT = pp.tile([128, 8, 128], BF16)
            nc.sync.dma_start(maskT[:], d["maskT"][:])

            x = [pp.tile([128, C], F32, name=f"x{b}", tag=f"x{b}")
                 for b in range(2)]
            kt_all = [[pp.tile([128, T], BF16, name=f"kt{b}{f}", tag=f"kt{b}{f}")
                       for f in range(NF)] for b in range(2)]
            v_aug = [pp.tile([128, 8, H, HD + 1], BF16, name=f"va{b}",
                             tag=f"va{b}") for b in range(2)]
            # own V, token-major, with the softmax-denominator ones column
            # baked in BEFORE the AllGather (so it ships with the data)
            v_oa = [pp.tile([128, H, HD + 1], BF16, name=f"voa{b}",
                            tag=f"voa{b}") for b in range(2)]
            for b in range(2):
                nc.vector.memset(v_oa[b][:, :, HD:HD + 1], 1.0)

            def layernorm(xt, tag):
                """One token tile [128, C] f32 -> bf16 normalized."""
                nm = sp.tile([128, 1], F32, name="nm", tag="nm")
                nc.vector.reduce_sum(nm[:], xt[:],
                                     axis=mybir.AxisListType.X, negate=True)
                nms = sp.tile([128, 1], F32, name="nms", tag="nms")
                nc.vector.tensor_scalar_mul(nms[:], nm[:], 1.0 / C)
                xc = ap_.tile([128, C], F32, name="xc", tag="xc", bufs=2)
                nc.vector.tensor_scalar_add(xc[:], xt[:], nms[:])
                sq = ap_.tile([128, C], BF16, name="sq", tag="sq", bufs=1)
                ssq = sp.tile([128, 1], F32, name="ssq", tag="ssq")
                nc.scalar.activation(sq[:], xc[:], AF.Square,
                                     bias=zt[:], accum_out=ssq[:])
                std = sp.tile([128, 1], F32, name="std", tag="std")
                nc.scalar.activation(std[:], ssq[:], AF.Sqrt,
                                     bias=eps[:], scale=1.0 / C)
                rstd = sp.tile([128, 1], F32, name="rstd", tag="rstd")
                nc.vector.reciprocal(rstd[:], std[:])
                hb = ap_.tile([128, C], BF16, name=f"h{tag}", tag=f"h{tag}")
                nc.vector.tensor_scalar_mul(hb[:], xc[:], rstd[:])
                return hb

            def transpose_128(src_ap, dst_ap, eng):
                ptr = psu.tile([128, 128], BF16, name="tr", tag="u")
                nc.tensor.transpose(ptr[:], src_ap, ident[:])
                if eng == 0:
                    nc.scalar.copy(dst_ap, ptr[:])
                else:
                    nc.vector.tensor_copy(dst_ap, ptr[:])

            def transpose_feat(h_b, tag):
                """h_b [128 tok, C] -> list of NF tiles [128 f, 128 tok]."""
                outs = []
                for f in range(NF):
                    t = ap_.tile([128, 128], BF16, name=f"{tag}{f}",
                                 tag=f"{tag}{f}", bufs=1)
                    transpose_128(h_b[:, f * 128:(f + 1) * 128], t[:], f % 2)
                    outs.append(t)
                return outs

            # weight tiles, loaded per layer (double-buffered)
            def load_weights(li):
                w = {}
                for nm, shp in (("aw", [128, NF, 3 * C]),
                                ("ala", [128, NF, R]),
                                ("alb", [R, 3 * C]),
                                ("pw", [128, NF, C]),
                                ("pla", [128, NF, R]),
                                ("plb", [R, C]),
                                ("fw", [128, NF, 4 * C]),
                                ("mw", [128, 16, C])):
                    w[nm] = wp.tile(shp, BF16, name=nm, tag=nm, bufs=2)
                    nc.sync.dma_start(w[nm][:], d[nm][li])
                return w

            # per-(layer,batch) attention state
            def qkv_block(w, b):
                """LN1, transposes, qkv matmul, fire the KV AllGather."""
                st = {}
                h = layernorm(x[b], f"1b{b}")
                hT = transpose_feat(h, f"hT{b}")
                pz = psu.tile([R, 128], F32, name="z", tag="u")
                for f in range(NF):
                    nc.tensor.matmul(pz[:], w["ala"][:, f, :], hT[f][:],
                                     start=(f == 0), stop=(f == NF - 1))
                z = sp.tile([R, 128], BF16, name="z", tag="z")
                nc.scalar.copy(z[:], pz[:])

                qT = ap_.tile([128, NF, 128], BF16, name=f"qT{b}",
                              tag=f"qT{b}")
                ktm = ap_.tile([128, NF, 128], BF16, name=f"ktm{b}",
                               tag=f"ktm{b}")
                vT = [ap_.tile([128, 128], BF16, name=f"vT{b}{f}",
                               tag=f"vT{b}{f}") for f in range(NF)]
                for chg in range(3):
                    pq = psu.tile([128, NF, 128], F32, name="mmq", tag="u")
                    for c4 in range(NF):
                        ch = chg * NF + c4
                        for f in range(NF):
                            nc.tensor.matmul(
                                pq[:, c4, :],
                                w["aw"][:, f, ch * 128:(ch + 1) * 128],
                                hT[f][:], start=(f == 0), stop=False)
                        nc.tensor.matmul(
                            pq[:, c4, :],
                            w["alb"][:, ch * 128:(ch + 1) * 128],
                            z[:], start=False, stop=True)
                    if chg == 0:
                        nc.vector.tensor_copy(qT[:], pq[:])
                    elif chg == 1:
                        nc.vector.tensor_copy(ktm[:], pq[:])
                    else:
                        for f in range(NF):
                            nc.vector.tensor_copy(vT[f][:], pq[:, f, :])
                # v token-major into the ones-augmented layout
                for f in range(NF):
                    ptr = psu.tile([128, 128], BF16, name="tr", tag="u")
                    nc.tensor.transpose(ptr[:], vT[f][:], ident[:])
                    cp = ptr[:].rearrange("p (a e) -> p a e", a=2)
                    if f % 2 == 0:
                        nc.scalar.copy(v_oa[b][:, 2 * f:2 * f + 2, 0:HD], cp)
                    else:
                        nc.vector.tensor_copy(
                            v_oa[b][:, 2 * f:2 * f + 2, 0:HD], cp)
                cin = dp.tile([2, 128, 8 * (HD + 1)], BF16, name=f"cin{b}",
                              tag=f"cin{b}")
                cout = dp.tile([8, 2, 128, 8 * (HD + 1)], BF16,
                               name=f"cout{b}", tag=f"cout{b}",
                               addr_space="Shared")
                nc.sync.dma_start(cin[0][:, 0:512],
                                  ktm[:].rearrange("p f k -> p (f k)"))
                nc.sync.dma_start(cin[1],
                                  v_oa[b][:].rearrange("p h e -> p (h e)"))
                nc.gpsimd.collective_compute(
                    "AllGather", mybir.AluOpType.bypass,
                    ins=[cin.opt()], outs=[cout.opt()],
                    replica_groups=[[0, 1, 2, 3, 4, 5, 6, 7]],
                )
                st["qT"] = qT
                st["cout"] = cout
                return st

            def attn_block(w, b, st):
                """Scatter AG result, scores+softmax+AV, proj, residual."""
                qT, cout = st["qT"], st["cout"]
                for f in range(NF):
                    nc.sync.dma_start(
                        kt_all[b][f][:].rearrange("p (r k) -> p r k", r=8),
                        cout[:, 0, :, f * 128:(f + 1) * 128].rearrange(
                            "r p k -> p r k"))
                nc.sync.dma_start(
                    v_aug[b][:].rearrange("p r h e -> p r (h e)"),
                    cout[:, 1, :, :].rearrange("r p c -> p r c"))

                y_sb = ap_.tile([128, C], BF16, name=f"y{b}", tag=f"y{b}")
                pts = {}
                for hh in range(H):
                    f, po = hh // 2, (hh % 2) * HD
                    for kpg in range(2):
                        pst = psu.tile([128, 4, 128], F32, name="mms",
                                       tag="u")
                        for j in range(4):
                            kc = kpg * 4 + j
                            nc.tensor.matmul(
                                pst[:, j, :],
                                kt_all[b][f][po:po + HD,
                                             kc * 128:(kc + 1) * 128],
                                qT[po:po + HD, f, :],
                                start=True, stop=True)
                        nc.vector.tensor_add(
                            pst[:], pst[:], maskT[:, kpg * 4:kpg * 4 + 4, :])
                        pt = ap3.tile([128, 4, 128], BF16, name="pt",
                                      tag="pt", bufs=9)
                        nc.scalar.activation(pt[:], pst[:], AF.Exp,
                                             bias=zt[:])
                        pts[(hh, kpg)] = pt
                    if hh % 4 == 3:
                        hb = hh // 4
                        pyav = psu.tile([128, 4, HD + 1], F32, name="yav",
                                        tag="u")
                        for hi in range(4):
                            h2 = hb * 4 + hi
                            for kb in range(8):
                                nc.tensor.matmul(
                                    pyav[:, hi, :],
                                    pts[(h2, kb // 4)][:, kb % 4, :],
                                    v_aug[b][:, kb, h2, :],
                                    start=(kb == 0), stop=(kb == 7))
                        sums = sp.tile([128, 4], F32, name="sums", tag="sums")
                        for j in range(4):
                            nc.scalar.copy(sums[:, j:j + 1],
                                           pyav[:, j, HD:HD + 1])
                        rec = sp.tile([128, 4], F32, name="rec", tag="rec")
                        nc.vector.reciprocal(rec[:], sums[:])
                        for j in range(4):
                            hj = hb * 4 + j
                            nc.vector.tensor_scalar_mul(
                                y_sb[:, hj * HD:(hj + 1) * HD],
                                pyav[:, j, 0:HD], rec[:, j:j + 1])

                # proj + LoRA + residual
                yT = transpose_feat(y_sb, f"yT{b}")
                pz2 = psu.tile([R, 128], F32, name="z", tag="u")
                for f in range(NF):
                    nc.tensor.matmul(pz2[:], w["pla"][:, f, :], yT[f][:],
                                     start=(f == 0), stop=(f == NF - 1))
                z2 = sp.tile([R, 128], BF16, name="z2", tag="z2")
                nc.scalar.copy(z2[:], pz2[:])
                pp_ = psu.tile([128, C], F32, name="mm512", tag="u")
                for f in range(NF):
                    nc.tensor.matmul(pp_[:], yT[f][:], w["pw"][:, f, :],
                                     start=(f == 0), stop=False)
                nc.tensor.matmul(pp_[:], z2[:], w["plb"][:],
                                 start=False, stop=True)
                nc.vector.tensor_add(x[b][:], x[b][:], pp_[:])

            def mlp_block(w, b, li):
                h2 = layernorm(x[b], f"2b{b}")
                h2T = transpose_feat(h2, f"h2T{b}")
                mF = ap_.tile([128, 16, 128], BF16, name=f"mF{b}",
                              tag=f"mF{b}")
                for gq in range(4):
                    pf = psu.tile([128, 4, 128], F32, name="mmf", tag="u")
                    for gi in range(4):
                        g = gq * 4 + gi
                        for f in range(NF):
                            nc.tensor.matmul(
                                pf[:, gi, :],
                                w["fw"][:, f, g * 128:(g + 1) * 128],
                                h2T[f][:],
                                start=(f == 0), stop=(f == NF - 1))
                    nc.scalar.activation(mF[:, gq * 4:gq * 4 + 4, :], pf[:],
                                         AF.Gelu_apprx_tanh, bias=zt[:])
                pm = psu.tile([128, C], F32, name="mm512", tag="u")
                for g in range(16):
                    nc.tensor.matmul(pm[:], mF[:, g, :], w["mw"][:, g, :],
                                     start=(g == 0), stop=(g == 15))
                nc.vector.tensor_add(x[b][:], x[b][:], pm[:])
                if debug:
                    nc.sync.dma_start(xdbg[li, b], x[b][:])

            # ---- prologue ----
            for b in range(2):
                nc.sync.dma_start(x[b][:], d["x0"][b])

            # ---- pipelined layers ----
            wts = [None, None]
            wts[0] = load_weights(0)
            st = [None, None]
            for li in range(L):
                w = wts[li % 2]
                st[0] = qkv_block(w, 0)
                if li > 0:
                    # deferred MLP of the other batch hides AG0's latency;
                    # it is the LAST reader of layer li-1's weights, so the
                    # li+1 prefetch (same slots) must be emitted after it.
                    mlp_block(wts[(li - 1) % 2], 1, li - 1)
                if li + 1 < L:
                    wts[(li + 1) % 2] = load_weights(li + 1)
                st[1] = qkv_block(w, 1)
                attn_block(w, 0, st[0])
                attn_block(w, 1, st[1])
                mlp_block(w, 0, li)
            mlp_block(wts[(L - 1) % 2], 1, L - 1)

            # ---- final LN + 8-rank AllGather of xfT + head ----
            xf = [layernorm(x[b], f"fb{b}") for b in range(2)]
            xfT = [ap_.tile([128, 256], BF16, name=f"xfT{f}", tag=f"xfT{f}")
                   for f in range(NF)]
            e = 0
            for b in range(2):
                for f in range(NF):
                    transpose_128(xf[b][:, f * 128:(f + 1) * 128],
                                  xfT[f][:, b * 128:(b + 1) * 128], e % 2)
                    e += 1
            cinF = dp.tile([NF, 128, 256], BF16, name="cinF", tag="cinF")
            coutF = dp.tile([8, NF, 128, 256], BF16, name="coutF",
                            tag="coutF", addr_space="Shared")
            for f in range(NF):
                nc.sync.dma_start(cinF[f], xfT[f][:])
            nc.gpsimd.collective_compute(
                "AllGather", mybir.AluOpType.bypass,
                ins=[cinF.opt()], outs=[coutF.opt()],
                replica_groups=[[0, 1, 2, 3, 4, 5, 6, 7]],
            )
            # reuse kt_all tiles (dead after layers) as gathered-xfT storage:
            # batch b tokens live in kt_all[b][f] [128, 1024]
            for f in range(NF):
                for b in range(2):
                    nc.sync.dma_start(
                        kt_all[b][f][:].rearrange("p (r q) -> p r q", r=8),
                        coutF[:, f, :, b * 128:(b + 1) * 128].rearrange(
                            "r p q -> p r q"))

            # head: out partition = vocab slice (128), free = tokens.
            for ch in range(13):
                nch = min(512, VCP - ch * 512)
                hwt = wp.tile([128, NF, 512], BF16, name="hw", tag="hw",
                              bufs=2)
                nc.sync.dma_start(
                    hwt[:, :, 0:nch],
                    d["hw"][:, ch * 512:ch * 512 + nch].rearrange(
                        "(f p) n -> p f n", p=128))
                for v in range(nch // 128):
                    stage = ap3.tile([128, 2 * T], BF16, name="lo",
                                     tag="lo", bufs=2)
                    for tg in range(4):
                        pl = psu.tile([128, 512], F32, name="mm512", tag="u")
                        for f in range(NF):
                            nc.tensor.matmul(
                                pl[:],
                                hwt[:, f, v * 128:(v + 1) * 128],
                                kt_all[tg // 2][f][:, (tg % 2) * 512:
                                                   (tg % 2) * 512 + 512],
                                start=(f == 0), stop=(f == NF - 1))
                        if tg % 2 == 0:
                            nc.scalar.copy(
                                stage[:, tg * 512:(tg + 1) * 512], pl[:])
                        else:
                            nc.vector.tensor_copy(
                                stage[:, tg * 512:(tg + 1) * 512], pl[:])
                    vg = ch * 4 + v
                    nc.sync.dma_start(
                        y_d[vg * 128:(vg + 1) * 128, :], stage[:])

    nc.compile()
    return nc


def _bf(a):
    return np.ascontiguousarray(a.astype(ml_dtypes.bfloat16))


def host_shards(inputs, debug=False):
    idx = np.asarray(inputs["idx"])
    wte = np.asarray(inputs["wte"], np.float32)
    wpe = np.asarray(inputs["wpe"], np.float32)
    ln1_g = np.asarray(inputs["ln1_g"], np.float32)
    ln2_g = np.asarray(inputs["ln2_g"], np.float32)
    lnf_g = np.asarray(inputs["lnf_g"], np.float32)
    for nm in ("ln1_b", "ln2_b", "fc_b", "mproj_b", "lnf_b"):
        assert np.abs(np.asarray(inputs[nm])).max() == 0.0, f"{nm} nonzero"
    LS = 32.0 / 8.0
    qs = 1.0 / math.sqrt(HD)

    aw = np.empty((L, C, 3 * C), np.float32)
    ala = np.empty((L, C, R), np.float32)
    alb = np.empty((L, R, 3 * C), np.float32)
    pw = np.empty((L, C, C), np.float32)
    pla = np.empty((L, C, R), np.float32)
    plb = np.empty((L, R, C), np.float32)
    fw = np.empty((L, C, 4 * C), np.float32)
    mw = np.empty((L, 4 * C, C), np.float32)
    for i in range(L):
        a = (np.asarray(inputs["attn_w"][i], np.float32) * ln1_g[i][None, :]).T
        a = a.copy()
        a[:, :C] *= qs
        aw[i] = a
        ala[i] = (np.asarray(inputs["attn_lA"][i], np.float32)
                  * ln1_g[i][None, :]).T
        b = np.asarray(inputs["attn_lB"][i], np.float32).T * LS
        b = b.copy()
        b[:, :C] *= qs
        alb[i] = b
        pw[i] = np.asarray(inputs["proj_w"][i], np.float32).T
        pla[i] = np.asarray(inputs["proj_lA"][i], np.float32).T
        plb[i] = np.asarray(inputs["proj_lB"][i], np.float32).T * LS
        fw[i] = (np.asarray(inputs["fc_w"][i], np.float32)
                 * ln2_g[i][None, :]).T
        mw[i] = np.asarray(inputs["mproj_w"][i], np.float32).T
    hwT = (np.asarray(inputs["head_w"], np.float32) * lnf_g[None, :]).T  # [C,V]

    def _reb(x, g):
        # [L, g*128, n] -> [L, 128, g, n] (kernel loads become contiguous)
        Lx, _, n = x.shape
        return np.ascontiguousarray(x.reshape(Lx, g, 128, n).swapaxes(1, 2))

    common = dict(aw=_bf(_reb(aw, NF)), ala=_bf(_reb(ala, NF)), alb=_bf(alb),
                  pw=_bf(_reb(pw, NF)), pla=_bf(_reb(pla, NF)), plb=_bf(plb),
                  fw=_bf(_reb(fw, NF)), mw=_bf(_reb(mw, 16)),
                  ident=_bf(np.eye(128, dtype=np.float32)))

    in_maps = []
    for c in range(NCORES):
        sl = slice(c * 128, (c + 1) * 128)
        x0 = np.stack([wte[idx[b2]][sl] + wpe[sl] for b2 in range(2)])
        x0 = np.ascontiguousarray(x0, np.float32)
        # maskT[kk, kc, qq]: 0 where key (kc*128+kk) <= query (c*128+qq)
        kidx = np.arange(128)[:, None, None] + 128 * np.arange(8)[None, :, None]
        qidx = c * 128 + np.arange(128)[None, None, :]
        maskT = _bf(np.where(kidx <= qidx, 0.0, NEG).astype(np.float32))
        hw = np.zeros((C, VCP), np.float32)
        lo, hi = c * VC, min((c + 1) * VC, V)
        hw[:, 0:hi - lo] = hwT[:, lo:hi]
        m = dict(common)
        m.update(x0=x0, maskT=maskT, hw=_bf(hw))
        in_maps.append(m)
    return in_maps


def kernel(**inputs):
    if "nc" not in _CACHE:
        _CACHE["nc"] = build_nc(debug=False)
    nc = _CACHE["nc"]
    in_maps = host_shards(inputs)
    res = bass_utils.run_bass_kernel_spmd(nc, in_maps,
                                          core_ids=list(range(NCORES)))
    out = np.empty((B * T, V), np.float32)
    for c in range(NCORES):
        lo, hi = c * VC, min((c + 1) * VC, V)
        out[:, lo:hi] = res.results[c]["y"][0:hi - lo, :].T.astype(np.float32)
    return out.reshape(B, T, V)


# revision 40
# speedup vs baseline: 1.0052x; 1.0052x over previous
"""GPT (4-layer, C=512, H=8, T=1024, B=2, V=50257, LoRA r=8) on 8 trn2 cores.

Sharding: every core owns global token tile c (128 tokens) of BOTH batches.
The two batch streams are software-pipelined inside each layer so the
per-batch 8-rank KV AllGather latency hides under the other batch's compute:
    qkv(i,b0) -> fire AG0 ; mlp(i-1,b1) ; qkv(i,b1) -> fire AG1 ;
    attn(i,b0) ; attn(i,b1) ; mlp(i,b0) ; [mlp(i,b1) deferred]
Head: vocab-sharded (6400-padded shard per core), out partition = vocab,
bf16 logits, 50 contiguous 512KB output DMAs; host upcasts/transposes.
"""
import math
import numpy as np
import ml_dtypes

import concourse.bass as bass
import concourse.bacc as bacc
import concourse.tile as tile
import concourse.mybir as mybir
from concourse import bass_utils

BF16 = mybir.dt.bfloat16
F32 = mybir.dt.float32
AF = mybir.ActivationFunctionType

L, H, C, V, B, T = 4, 8, 512, 50257, 2, 1024
R = 8
NCORES = 8
NF = C // 128        # 4 feature tiles
HD = C // H          # 64 head dim
VC = 6283            # true vocab shard (8*6283 = 50264 >= 50257)
VCP = 6400           # padded shard: 50 slices of 128
NEG = -1.0e9

_CACHE = {}


def build_nc(debug=False):
    nc = bacc.Bacc("TRN2", target_bir_lowering=False, debug=False,
                   num_devices=NCORES)
    d = {}
    def inp(name, shape, dt):
        d[name] = nc.dram_tensor(name, shape, dt, kind="ExternalInput").ap()
    inp("x0", [2, 128, C], F32)        # [batch, own 128 tokens, C]
    inp("maskT", [128, 8, 128], BF16)  # [kk, kc, qq] causal add-mask
    inp("ident", [128, 128], BF16)
    # weights host-pre-rearranged to [p, f, n] so loads are contiguous
    inp("aw", [L, 128, NF, 3 * C], BF16)
    inp("ala", [L, 128, NF, R], BF16)
    inp("alb", [L, R, 3 * C], BF16)    # *4.0, q-cols pre-scaled
    inp("pw", [L, 128, NF, C], BF16)
    inp("pla", [L, 128, NF, R], BF16)
    inp("plb", [L, R, C], BF16)        # *4.0
    inp("fw", [L, 128, NF, 4 * C], BF16)
    inp("mw", [L, 128, 16, C], BF16)
    inp("hw", [C, VCP], BF16)          # head shard (rank-dep, zero-padded)
    y_d = nc.dram_tensor("y", [VCP, 2 * T], BF16, kind="ExternalOutput").ap()
    if debug:
        xdbg = nc.dram_tensor("xdbg", [L, 2, 128, C], F32,
                              kind="ExternalOutput").ap()

    with tile.TileContext(nc) as tc:
        with (
            tc.tile_pool(name="persist", bufs=1) as pp,
            tc.tile_pool(name="wts", bufs=1) as wp,
            tc.tile_pool(name="acts", bufs=1) as ap_,
            tc.tile_pool(name="acts3", bufs=3) as ap3,
            tc.tile_pool(name="stats", bufs=3) as sp,
            tc.tile_pool(name="dram", bufs=2, space="DRAM") as dp,
            tc.tile_pool(name="psu", bufs=8, space="PSUM") as psu,
        ):
            ident = pp.tile([128, 128], BF16)
            nc.sync.dma_start(ident[:], d["ident"][:])
            zt = pp.tile([128, 1], F32)
            nc.vector.memset(zt[:], 0.0)
            eps = pp.tile([128, 1], F32)
            nc.vector.memset(eps[:], 1e-5)
            maskT = pp.tile([128, 8, 128], BF16)
            nc.sync.dma_start(maskT[:], d["maskT"][:])

            x = [pp.tile([128, C], F32, name=f"x{b}", tag=f"x{b}")
                 for b in range(2)]
            # flat [p, 4096]; layers view it [p, rank, f, k], the head reuses
            # it as gathered-token storage [p, r4, f, b, q]
            kt_all = [pp.tile([128, 8 * NF * 128], BF16, name=f"kt{b}",
                              tag=f"kt{b}") for b in range(2)]
            v_aug = [pp.tile([128, 8, H, HD + 1], BF16, name=f"va{b}",
                             tag=f"va{b}") for b in range(2)]
            # own V, token-major, with the softmax-denominator ones column
            # baked in BEFORE the AllGather (so it ships with the data)
            v_oa = [pp.tile([128, H, HD + 1], BF16, name=f"voa{b}",
                            tag=f"voa{b}") for b in range(2)]
            for b in range(2):
                nc.vector.memset(v_oa[b][:, :, HD:HD + 1], 1.0)

            def layernorm(xt, tag):
                """One token tile [128, C] f32 -> bf16 normalized."""
                nm = sp.tile([128, 1], F32, name="nm", tag="nm")
                nc.vector.reduce_sum(nm[:], xt[:],
                                     axis=mybir.AxisListType.X, negate=True)
                nms = sp.tile([128, 1], F32, name="nms", tag="nms")
                nc.vector.tensor_scalar_mul(nms[:], nm[:], 1.0 / C)
                xc = ap_.tile([128, C], F32, name="xc", tag="xc", bufs=2)
                nc.vector.tensor_scalar_add(xc[:], xt[:], nms[:])
                sq = ap_.tile([128, C], BF16, name="sq", tag="sq", bufs=1)
                ssq = sp.tile([128, 1], F32, name="ssq", tag="ssq")
                nc.scalar.activation(sq[:], xc[:], AF.Square,
                                     bias=zt[:], accum_out=ssq[:])
                std = sp.tile([128, 1], F32, name="std", tag="std")
                nc.scalar.activation(std[:], ssq[:], AF.Sqrt,
                                     bias=eps[:], scale=1.0 / C)
                rstd = sp.tile([128, 1], F32, name="rstd", tag="rstd")
                nc.vector.reciprocal(rstd[:], std[:])
                hb = ap_.tile([128, C], BF16, name=f"h{tag}", tag=f"h{tag}")
                nc.vector.tensor_scalar_mul(hb[:], xc[:], rstd[:])
                return hb

            def transpose_128(src_ap, dst_ap, eng):
                ptr = psu.tile([128, 128], BF16, name="tr", tag="u")
                nc.tensor.transpose(ptr[:], src_ap, ident[:])
                if eng == 0:
                    nc.scalar.copy(dst_ap, ptr[:])
                else:
                    nc.vector.tensor_copy(dst_ap, ptr[:])

            def transpose_feat(h_b, tag):
                """h_b [128 tok, C] -> list of NF tiles [128 f, 128 tok]."""
                outs = []
                for f in range(NF):
                    t = ap_.tile([128, 128], BF16, name=f"{tag}{f}",
                                 tag=f"{tag}{f}", bufs=1)
                    transpose_128(h_b[:, f * 128:(f + 1) * 128], t[:], f % 2)
                    outs.append(t)
                return outs

            # weight tiles, loaded per layer (double-buffered)
            def load_weights(li):
                w = {}
                for nm, shp in (("aw", [128, NF, 3 * C]),
                                ("ala", [128, NF, R]),
                                ("alb", [R, 3 * C]),
                                ("pw", [128, NF, C]),
                                ("pla", [128, NF, R]),
                                ("plb", [R, C]),
                                ("fw", [128, NF, 4 * C]),
                                ("mw", [128, 16, C])):
                    w[nm] = wp.tile(shp, BF16, name=nm, tag=nm, bufs=2)
                    nc.sync.dma_start(w[nm][:], d[nm][li])
                return w

            # per-(layer,batch) attention state
            def qkv_block(w, b):
                """LN1, transposes, qkv matmul, fire the KV AllGather."""
                st = {}
                h = layernorm(x[b], f"1b{b}")
                hT = transpose_feat(h, f"hT{b}")
                pz = psu.tile([R, 128], F32, name="z", tag="u")
                for f in range(NF):
                    nc.tensor.matmul(pz[:], w["ala"][:, f, :], hT[f][:],
                                     start=(f == 0), stop=(f == NF - 1))
                z = sp.tile([R, 128], BF16, name="z", tag="z")
                nc.scalar.copy(z[:], pz[:])

                qT = ap_.tile([128, NF, 128], BF16, name=f"qT{b}",
                              tag=f"qT{b}")
                ktm = ap_.tile([128, NF, 128], BF16, name=f"ktm{b}",
                               tag=f"ktm{b}")
                vT = [ap_.tile([128, 128], BF16, name=f"vT{b}{f}",
                               tag=f"vT{b}{f}") for f in range(NF)]
                for chg in range(3):
                    pq = psu.tile([128, NF, 128], F32, name="mmq", tag="u")
                    for c4 in range(NF):
                        ch = chg * NF + c4
                        for f in range(NF):
                            nc.tensor.matmul(
                                pq[:, c4, :],
                                w["aw"][:, f, ch * 128:(ch + 1) * 128],
                                hT[f][:], start=(f == 0), stop=False)
                        nc.tensor.matmul(
                            pq[:, c4, :],
                            w["alb"][:, ch * 128:(ch + 1) * 128],
                            z[:], start=False, stop=True)
                    if chg == 0:
                        nc.vector.tensor_copy(qT[:], pq[:])
                    elif chg == 1:
                        nc.vector.tensor_copy(ktm[:], pq[:])
                    else:
                        for f in range(NF):
                            nc.vector.tensor_copy(vT[f][:], pq[:, f, :])
                # v token-major into the ones-augmented layout
                for f in range(NF):
                    ptr = psu.tile([128, 128], BF16, name="tr", tag="u")
                    nc.tensor.transpose(ptr[:], vT[f][:], ident[:])
                    cp = ptr[:].rearrange("p (a e) -> p a e", a=2)
                    if f % 2 == 0:
                        nc.scalar.copy(v_oa[b][:, 2 * f:2 * f + 2, 0:HD], cp)
                    else:
                        nc.vector.tensor_copy(
                            v_oa[b][:, 2 * f:2 * f + 2, 0:HD], cp)
                cin = dp.tile([2, 128, 8 * (HD + 1)], BF16, name=f"cin{b}",
                              tag=f"cin{b}")
                cout = dp.tile([8, 2, 128, 8 * (HD + 1)], BF16,
                               name=f"cout{b}", tag=f"cout{b}",
                               addr_space="Shared")
                nc.sync.dma_start(cin[0][:, 0:512],
                                  ktm[:].rearrange("p f k -> p (f k)"))
                nc.sync.dma_start(cin[1],
                                  v_oa[b][:].rearrange("p h e -> p (h e)"))
                nc.gpsimd.collective_compute(
                    "AllGather", mybir.AluOpType.bypass,
                    ins=[cin.opt()], outs=[cout.opt()],
                    replica_groups=[[0, 1, 2, 3, 4, 5, 6, 7]],
                )
                st["qT"] = qT
                st["cout"] = cout
                return st

            def attn_block(w, b, st):
                """Scatter AG result, scores+softmax+AV, proj, residual."""
                qT, cout = st["qT"], st["cout"]
                nc.sync.dma_start(
                    kt_all[b][:].rearrange("p (r c) -> p r c", r=8),
                    cout[:, 0, :, 0:512].rearrange("r p c -> p r c"))
                ktv = kt_all[b][:].rearrange("p (r f k) -> p r f k",
                                             r=8, f=NF)
                for rh in range(2):
                    nc.sync.dma_start(
                        v_aug[b][:, rh * 4:(rh + 1) * 4, :, :].rearrange(
                            "p r h e -> p r (h e)"),
                        cout[rh * 4:(rh + 1) * 4, 1, :, :].rearrange(
                            "r p c -> p r c"))

                y_sb = ap_.tile([128, C], BF16, name=f"y{b}", tag=f"y{b}")
                pts = {}
                for hh in range(H):
                    f, po = hh // 2, (hh % 2) * HD
                    for kpg in range(2):
                        pst = psu.tile([128, 4, 128], F32, name="mms",
                                       tag="u")
                        for j in range(4):
                            kc = kpg * 4 + j
                            nc.tensor.matmul(
                                pst[:, j, :],
                                ktv[po:po + HD, kc, f, :],
                                qT[po:po + HD, f, :],
                                start=True, stop=True)
                        nc.vector.tensor_add(
                            pst[:], pst[:], maskT[:, kpg * 4:kpg * 4 + 4, :])
                        pt = ap3.tile([128, 4, 128], BF16, name="pt",
                                      tag="pt", bufs=9)
                        nc.scalar.activation(pt[:], pst[:], AF.Exp,
                                             bias=zt[:])
                        pts[(hh, kpg)] = pt
                    if hh % 4 == 3:
                        hb = hh // 4
                        pyav = psu.tile([128, 4, HD + 1], F32, name="yav",
                                        tag="u")
                        for hi in range(4):
                            h2 = hb * 4 + hi
                            for kb in range(8):
                                nc.tensor.matmul(
                                    pyav[:, hi, :],
                                    pts[(h2, kb // 4)][:, kb % 4, :],
                                    v_aug[b][:, kb, h2, :],
                                    start=(kb == 0), stop=(kb == 7))
                        sums = sp.tile([128, 4], F32, name="sums", tag="sums")
                        for j in range(4):
                            nc.scalar.copy(sums[:, j:j + 1],
                                           pyav[:, j, HD:HD + 1])
                        rec = sp.tile([128, 4], F32, name="rec", tag="rec")
                        nc.vector.reciprocal(rec[:], sums[:])
                        for j in range(4):
                            hj = hb * 4 + j
                            nc.vector.tensor_scalar_mul(
                                y_sb[:, hj * HD:(hj + 1) * HD],
                                pyav[:, j, 0:HD], rec[:, j:j + 1])

                # proj + LoRA + residual
                yT = transpose_feat(y_sb, f"yT{b}")
                pz2 = psu.tile([R, 128], F32, name="z", tag="u")
                for f in range(NF):
                    nc.tensor.matmul(pz2[:], w["pla"][:, f, :], yT[f][:],
                                     start=(f == 0), stop=(f == NF - 1))
                z2 = sp.tile([R, 128], BF16, name="z2", tag="z2")
                nc.scalar.copy(z2[:], pz2[:])
                pp_ = psu.tile([128, C], F32, name="mm512", tag="u")
                for f in range(NF):
                    nc.tensor.matmul(pp_[:], yT[f][:], w["pw"][:, f, :],
                                     start=(f == 0), stop=False)
                nc.tensor.matmul(pp_[:], z2[:], w["plb"][:],
                                 start=False, stop=True)
                nc.vector.tensor_add(x[b][:], x[b][:], pp_[:])

            def mlp_block(w, b, li):
                h2 = layernorm(x[b], f"2b{b}")
                h2T = transpose_feat(h2, f"h2T{b}")
                mF = ap_.tile([128, 16, 128], BF16, name=f"mF{b}",
                              tag=f"mF{b}")
                for gq in range(4):
                    pf = psu.tile([128, 4, 128], F32, name="mmf", tag="u")
                    for gi in range(4):
                        g = gq * 4 + gi
                        for f in range(NF):
                            nc.tensor.matmul(
                                pf[:, gi, :],
                                w["fw"][:, f, g * 128:(g + 1) * 128],
                                h2T[f][:],
                                start=(f == 0), stop=(f == NF - 1))
                    nc.scalar.activation(mF[:, gq * 4:gq * 4 + 4, :], pf[:],
                                         AF.Gelu_apprx_tanh, bias=zt[:])
                pm = psu.tile([128, C], F32, name="mm512", tag="u")
                for g in range(16):
                    nc.tensor.matmul(pm[:], mF[:, g, :], w["mw"][:, g, :],
                                     start=(g == 0), stop=(g == 15))
                nc.vector.tensor_add(x[b][:], x[b][:], pm[:])
                if debug:
                    nc.sync.dma_start(xdbg[li, b], x[b][:])

            # ---- prologue ----
            for b in range(2):
                nc.sync.dma_start(x[b][:], d["x0"][b])

            # ---- pipelined layers ----
            wts = [None, None]
            wts[0] = load_weights(0)
            st = [None, None]
            for li in range(L):
                w = wts[li % 2]
                st[0] = qkv_block(w, 0)
                if li > 0:
                    # deferred MLP of the other batch hides AG0's latency;
                    # it is the LAST reader of layer li-1's weights, so the
                    # li+1 prefetch (same slots) must be emitted after it.
                    mlp_block(wts[(li - 1) % 2], 1, li - 1)
                if li + 1 < L:
                    wts[(li + 1) % 2] = load_weights(li + 1)
                st[1] = qkv_block(w, 1)
                attn_block(w, 0, st[0])
                attn_block(w, 1, st[1])
                mlp_block(w, 0, li)
            mlp_block(wts[(L - 1) % 2], 1, L - 1)

            # ---- final LN + 8-rank AllGather of xfT + head ----
            xf = [layernorm(x[b], f"fb{b}") for b in range(2)]
            xfB = ap_.tile([128, NF, 2, 128], BF16, name="xfB", tag="xfB")
            e = 0
            for b in range(2):
                for f in range(NF):
                    transpose_128(xf[b][:, f * 128:(f + 1) * 128],
                                  xfB[:, f, b, :], e % 2)
                    e += 1
            cinF = dp.tile([128, NF * 2 * 128], BF16, name="cinF", tag="cinF")
            coutF = dp.tile([8, 128, NF * 2 * 128], BF16, name="coutF",
                            tag="coutF", addr_space="Shared")
            nc.sync.dma_start(cinF[:],
                              xfB[:].rearrange("p f b q -> p (f b q)"))
            nc.gpsimd.collective_compute(
                "AllGather", mybir.AluOpType.bypass,
                ins=[cinF.opt()], outs=[coutF.opt()],
                replica_groups=[[0, 1, 2, 3, 4, 5, 6, 7]],
            )
            # reuse kt_all tiles (dead after layers) as gathered-xf storage:
            # kt_all[g] holds ranks 4g..4g+3, flat (rank, f, b, q) per row
            for g in range(2):
                nc.sync.dma_start(
                    kt_all[g][:].rearrange("p (a c) -> p a c", a=4),
                    coutF[g * 4:(g + 1) * 4].rearrange("r p c -> p r c"))
            xv = [kt_all[g][:].rearrange("p (a f b q) -> p a f b q",
                                         a=4, f=NF, b=2) for g in range(2)]

            # head: out partition = vocab slice (128), free = tokens.
            for ch in range(13):
                nch = min(512, VCP - ch * 512)
                hwt = wp.tile([128, NF, 512], BF16, name="hw", tag="hw",
                              bufs=2)
                nc.sync.dma_start(
                    hwt[:, :, 0:nch],
                    d["hw"][:, ch * 512:ch * 512 + nch].rearrange(
                        "(f p) n -> p f n", p=128))
                for v in range(nch // 128):
                    stage = ap3.tile([128, 2 * T], BF16, name="lo",
                                     tag="lo", bufs=2)
                    for tg in range(4):
                        pl = psu.tile([128, 512], F32, name="mm512", tag="u")
                        for f in range(NF):
                            nc.tensor.matmul(
                                pl[:],
                                hwt[:, f, v * 128:(v + 1) * 128],
                                xv[tg % 2][:, :, f, tg // 2, :],
                                start=(f == 0), stop=(f == NF - 1))
                        if tg % 2 == 0:
                            nc.scalar.copy(
                                stage[:, tg * 512:(tg + 1) * 512], pl[:])
                        else:
                            nc.vector.tensor_copy(
                                stage[:, tg * 512:(tg + 1) * 512], pl[:])
                    vg = ch * 4 + v
                    nc.sync.dma_start(
                        y_d[vg * 128:(vg + 1) * 128, :], stage[:])

    nc.compile()
    return nc


def _bf(a):
    return np.ascontiguousarray(a.astype(ml_dtypes.bfloat16))


def host_shards(inputs, debug=False):
    idx = np.asarray(inputs["idx"])
    wte = np.asarray(inputs["wte"], np.float32)
    wpe = np.asarray(inputs["wpe"], np.float32)
    ln1_g = np.asarray(inputs["ln1_g"], np.float32)
    ln2_g = np.asarray(inputs["ln2_g"], np.float32)
    lnf_g = np.asarray(inputs["lnf_g"], np.float32)
    for nm in ("ln1_b", "ln2_b", "fc_b", "mproj_b", "lnf_b"):
        assert np.abs(np.asarray(inputs[nm])).max() == 0.0, f"{nm} nonzero"
    LS = 32.0 / 8.0
    qs = 1.0 / math.sqrt(HD)

    aw = np.empty((L, C, 3 * C), np.float32)
    ala = np.empty((L, C, R), np.float32)
    alb = np.empty((L, R, 3 * C), np.float32)
    pw = np.empty((L, C, C), np.float32)
    pla = np.empty((L, C, R), np.float32)
    plb = np.empty((L, R, C), np.float32)
    fw = np.empty((L, C, 4 * C), np.float32)
    mw = np.empty((L, 4 * C, C), np.float32)
    for i in range(L):
        a = (np.asarray(inputs["attn_w"][i], np.float32) * ln1_g[i][None, :]).T
        a = a.copy()
        a[:, :C] *= qs
        aw[i] = a
        ala[i] = (np.asarray(inputs["attn_lA"][i], np.float32)
                  * ln1_g[i][None, :]).T
        b = np.asarray(inputs["attn_lB"][i], np.float32).T * LS
        b = b.copy()
        b[:, :C] *= qs
        alb[i] = b
        pw[i] = np.asarray(inputs["proj_w"][i], np.float32).T
        pla[i] = np.asarray(inputs["proj_lA"][i], np.float32).T
        plb[i] = np.asarray(inputs["proj_lB"][i], np.float32).T * LS
        fw[i] = (np.asarray(inputs["fc_w"][i], np.float32)
                 * ln2_g[i][None, :]).T
        mw[i] = np.asarray(inputs["mproj_w"][i], np.float32).T
    hwT = (np.asarray(inputs["head_w"], np.float32) * lnf_g[None, :]).T  # [C,V]

    def _reb(x, g):
        # [L, g*128, n] -> [L, 128, g, n] (kernel loads become contiguous)
        Lx, _, n = x.shape
        return np.ascontiguousarray(x.reshape(Lx, g, 128, n).swapaxes(1, 2))

    common = dict(aw=_bf(_reb(aw, NF)), ala=_bf(_reb(ala, NF)), alb=_bf(alb),
                  pw=_bf(_reb(pw, NF)), pla=_bf(_reb(pla, NF)), plb=_bf(plb),
                  fw=_bf(_reb(fw, NF)), mw=_bf(_reb(mw, 16)),
                  ident=_bf(np.eye(128, dtype=np.float32)))

    in_maps = []
    for c in range(NCORES):
        sl = slice(c * 128, (c + 1) * 128)
        x0 = np.stack([wte[idx[b2]][sl] + wpe[sl] for b2 in range(2)])
        x0 = np.ascontiguousarray(x0, np.float32)
        # maskT[kk, kc, qq]: 0 where key (kc*128+kk) <= query (c*128+qq)
        kidx = np.arange(128)[:, None, None] + 128 * np.arange(8)[None, :, None]
        qidx = c * 128 + np.arange(128)[None, None, :]
        maskT = _bf(np.where(kidx <= qidx, 0.0, NEG).astype(np.float32))
        hw = np.zeros((C, VCP), np.float32)
        lo, hi = c * VC, min((c + 1) * VC, V)
        hw[:, 0:hi - lo] = hwT[:, lo:hi]
        m = dict(common)
        m.update(x0=x0, maskT=maskT, hw=_bf(hw))
        in_maps.append(m)
    return in_maps


def kernel(**inputs):
    if "nc" not in _CACHE:
        _CACHE["nc"] = build_nc(debug=False)
    nc = _CACHE["nc"]
    in_maps = host_shards(inputs)
    res = bass_utils.run_bass_kernel_spmd(nc, in_maps,
                                          core_ids=list(range(NCORES)))
    out = np.empty((B * T, V), np.float32)
    for c in range(NCORES):
        lo, hi = c * VC, min((c + 1) * VC, V)
        out[:, lo:hi] = res.results[c]["y"][0:hi - lo, :].T.astype(np.float32)
    return out.reshape(B, T, V)
